# revision 7
# baseline (speedup 1.0000x reference)
"""Bidirectional 2-layer LSTM encoder (nn_Encoder) on 8 Trainium2 NeuronCores.

Strategy
--------
- 4 batch slices x 2 directions = 8 cores. Core 2i runs the forward
  direction for batch slice i, core 2i+1 the backward direction.
- The backward direction over a ragged batch is mathematically equivalent to
  a forward scan over the globally time-reversed sequence with the validity
  mask applied (state frozen while masked), so one direction-agnostic device
  program serves all cores; the host prepares per-core scan-ordered inputs.
- State freezing is done arithmetically: a K=1 matmul adds -25/+25 times the
  per-token mask flag to the i/f gate pre-activations (sigmoid saturates to
  0/1 to ~1e-9), so the scan itself has no mask/select ops.
- Per layer the device runs a 128-step recurrent scan in a transposed layout
  (gates/state as [128, batch] tiles, PE weight-load bound), with the batched
  input matmul (Gin) chunk-interleaved into the scan so Gin matmuls fill the
  PE gaps left by each step's elementwise dependency chain. Gin chunks live
  in SBUF (no DRAM round-trip).
- Layers are two launches of the same compiled program; the host reshuffles
  h0 (direction un-reversal + concat) between launches and assembles/masks
  the final output.
"""

import os
import numpy as np

S, M, V, E, H = 128, 256, 32000, 1024, 512
PAD_ID = V - 1
N_CORES = 8
B = 64          # batch columns per core (M / 4 pairs)
KT = 8          # k-tiles of the 1024-dim input
HT = 4          # k-tiles of the 512-dim hidden
MT = 16         # m-tiles of the 4H=2048 gate dim
SPC = 4         # scan steps covered per Gin chunk
FD = SPC * B    # Gin matmul free-dim chunk (256)
NCH = S // SPC  # 32 chunks
FORCE = 25.0    # gate-forcing magnitude for masked steps

_cache = {}

# Populated with [exec_time_ns_layer0, exec_time_ns_layer1] when tracing.
last_exec_ns = []
last_trace_dirs = []


def _build_program():
    import concourse.bacc as bacc
    import concourse.tile as tile
    from concourse import mybir

    f32 = mybir.dt.float32
    bf16 = mybir.dt.bfloat16
    Sig = mybir.ActivationFunctionType.Sigmoid
    Tanh = mybir.ActivationFunctionType.Tanh
    add = mybir.AluOpType.add
    mult = mybir.AluOpType.mult

    nc = bacc.Bacc("TRN2", target_bir_lowering=False, debug=False, num_devices=1)

    xt = nc.dram_tensor("xt", [E, S * B], bf16, kind="ExternalInput").ap()
    wih = nc.dram_tensor("wih", [E, 4 * H], bf16, kind="ExternalInput").ap()
    whh = nc.dram_tensor("whh", [H, 4 * H], bf16, kind="ExternalInput").ap()
    bias = nc.dram_tensor("bias", [128, MT], f32, kind="ExternalInput").ap()
    wm = nc.dram_tensor("wm", [1, 1024], bf16, kind="ExternalInput").ap()
    mk = nc.dram_tensor("mk", [1, S * B], bf16, kind="ExternalInput").ap()
    hout = nc.dram_tensor("hout", [S, HT, 128, B], f32, kind="ExternalOutput").ap()

    with tile.TileContext(nc) as tc:
        with (
            tc.tile_pool(name="wp", bufs=1) as wp,
            tc.tile_pool(name="xp", bufs=24) as xp,
            tc.tile_pool(name="gps", bufs=4, space="PSUM") as gps,
            tc.tile_pool(name="gch", bufs=3) as gchp,
            tc.tile_pool(name="sps", bufs=4, space="PSUM") as sps,
            tc.tile_pool(name="gsum", bufs=4) as gsump,
            tc.tile_pool(name="act", bufs=2) as actp,
            tc.tile_pool(name="st", bufs=2) as stp,
            tc.tile_pool(name="tmp", bufs=2) as tmpp,
        ):
            # ---- resident weights ----
            wih_sb = wp.tile([128, KT * 4 * H], bf16, tag="wih")
            for k in range(KT):
                nc.sync.dma_start(
                    wih_sb[:, k * 4 * H : (k + 1) * 4 * H],
                    wih[k * 128 : (k + 1) * 128, :],
                )
            whh_sb = wp.tile([128, HT * 4 * H], bf16, tag="whh")
            for k in range(HT):
                nc.sync.dma_start(
                    whh_sb[:, k * 4 * H : (k + 1) * 4 * H],
                    whh[k * 128 : (k + 1) * 128, :],
                )
            bias_sb = wp.tile([128, MT], f32, tag="bias")
            nc.sync.dma_start(bias_sb[:], bias[:])
            wm_sb = wp.tile([1, 1024], bf16, tag="wm")
            nc.sync.dma_start(wm_sb[:], wm[:])
            mk_sb = wp.tile([1, S * B], bf16, tag="mk")
            nc.sync.dma_start(mk_sb[:], mk[:])

            xtiles = {}   # chunk -> list of 8 sbuf tiles
            gchunks = {}  # chunk -> sbuf tile [128, MT*FD]

            def emit_xdma(c):
                ts = []
                for k in range(KT):
                    x_t = xp.tile([128, FD], bf16, tag="x")
                    nc.sync.dma_start(
                        x_t[:], xt[k * 128 : (k + 1) * 128, c * FD : (c + 1) * FD]
                    )
                    ts.append(x_t)
                xtiles[c] = ts

            def emit_gin_quarter(c, q):
                # compute Gin for chunk c, m-tiles 4q..4q+3
                if q == 0:
                    gchunks[c] = gchp.tile(
                        [128, MT * FD], f32, tag="gch", name=f"gch{c}"
                    )
                gt = gchunks[c]
                xts = xtiles[c]
                for m in range(4 * q, 4 * q + 4):
                    ps = gps.tile([128, FD], f32, tag="gps")
                    for k in range(KT):
                        nc.tensor.matmul(
                            out=ps[:],
                            lhsT=wih_sb[:, k * 4 * H + m * 128 : k * 4 * H + (m + 1) * 128],
                            rhs=xts[k][:],
                            start=(k == 0),
                            stop=(k == KT - 1 and m >= 8),
                        )
                    if m < 8:
                        # mask forcing on i/f gates
                        nc.tensor.matmul(
                            out=ps[:],
                            lhsT=wm_sb[:, m * 128 : (m + 1) * 128],
                            rhs=mk_sb[:, c * FD : (c + 1) * FD],
                            start=False,
                            stop=True,
                        )
                    nc.vector.tensor_scalar(
                        out=gt[:, m * FD : (m + 1) * FD], in0=ps[:],
                        scalar1=bias_sb[:, m : m + 1], scalar2=None, op0=add,
                    )

            # ---- scan state ----
            c_prev = stp.tile([128, 4 * B], f32, tag="c")
            nc.vector.memset(c_prev[:], 0.0)
            hbf_prev = stp.tile([128, 4 * B], bf16, tag="hbf")
            nc.vector.memset(hbf_prev[:], 0.0)

            def emit_scan_step(s):
                nonlocal c_prev, hbf_prev
                blk, so = s // SPC, s % SPC
                gt = gchunks[blk]
                # gin view: [128, m, so, j] -> per-step slice [128, m-range, j]
                gv = gt[:].rearrange("p (m s j) -> p m s j", m=MT, s=SPC)
                psA = sps.tile([128, 8 * B], f32, tag="ps")
                psB = sps.tile([128, 8 * B], f32, tag="ps")
                for m in range(MT):
                    ps = psA if m < 8 else psB
                    col = (m % 8) * B
                    for k in range(HT):
                        nc.tensor.matmul(
                            out=ps[:, col : col + B],
                            lhsT=whh_sb[:, k * 4 * H + m * 128 : k * 4 * H + (m + 1) * 128],
                            rhs=hbf_prev[:, k * B : (k + 1) * B],
                            start=(k == 0),
                            stop=(k == HT - 1),
                        )
                gsA = gsump.tile([128, 8 * B], f32, tag="gs")
                gsB = gsump.tile([128, 8 * B], f32, tag="gs")
                nc.vector.tensor_tensor(
                    out=gsA[:].rearrange("p (m j) -> p m j", m=8),
                    in0=psA[:].rearrange("p (m j) -> p m j", m=8),
                    in1=gv[:, 0:8, so, :], op=add,
                )
                nc.vector.tensor_tensor(
                    out=gsB[:].rearrange("p (m j) -> p m j", m=8),
                    in0=psB[:].rearrange("p (m j) -> p m j", m=8),
                    in1=gv[:, 8:16, so, :], op=add,
                )

                si = actp.tile([128, 4 * B], f32, tag="si")
                nc.scalar.activation(si[:], gsA[:, : 4 * B], Sig)
                sf = actp.tile([128, 4 * B], f32, tag="sf")
                nc.scalar.activation(sf[:], gsA[:, 4 * B :], Sig)
                tg = actp.tile([128, 4 * B], f32, tag="tg")
                nc.scalar.activation(tg[:], gsB[:, : 4 * B], Tanh)
                so_ = actp.tile([128, 4 * B], f32, tag="so")
                nc.scalar.activation(so_[:], gsB[:, 4 * B :], Sig)

                u = tmpp.tile([128, 4 * B], f32, tag="u")
                nc.gpsimd.tensor_tensor(out=u[:], in0=si[:], in1=tg[:], op=mult)
                cf = tmpp.tile([128, 4 * B], f32, tag="cf")
                nc.vector.tensor_tensor(out=cf[:], in0=sf[:], in1=c_prev[:], op=mult)
                c_new = stp.tile([128, 4 * B], f32, tag="c")
                nc.vector.tensor_tensor(out=c_new[:], in0=cf[:], in1=u[:], op=add)
                tcn = tmpp.tile([128, 4 * B], f32, tag="tcn")
                nc.scalar.activation(tcn[:], c_new[:], Tanh)
                hbf_new = stp.tile([128, 4 * B], bf16, tag="hbf")
                nc.vector.tensor_tensor(out=hbf_new[:], in0=so_[:], in1=tcn[:], op=mult)
                h_new = tmpp.tile([128, 4 * B], f32, tag="h")
                nc.gpsimd.tensor_tensor(out=h_new[:], in0=so_[:], in1=tcn[:], op=mult)

                dst = hout[s, :, :, :].rearrange("k p j -> p k j")
                nc.sync.dma_start(dst, h_new[:].rearrange("p (k j) -> p k j", k=HT))

                c_prev = c_new
                hbf_prev = hbf_new

            # ---- emission: prologue then interleaved steady state ----
            emit_xdma(0)
            emit_xdma(1)
            emit_xdma(2)
            for c in (0, 1):
                for q in range(4):
                    emit_gin_quarter(c, q)
            for s in range(S):
                emit_scan_step(s)
                blk, q = s // SPC, s % SPC
                cc = blk + 2
                if cc < NCH:
                    if q == 0 and cc + 1 < NCH:
                        emit_xdma(cc + 1)
                    emit_gin_quarter(cc, q)

    nc.compile()
    return nc


def _get_program():
    if "nc" not in _cache:
        _cache["nc"] = _build_program()
    return _cache["nc"]


def _prep_weights(Wih, Whh, b, bf16):
    """Per-direction device weight tensors."""
    out = []
    for d in range(2):
        wihT = np.ascontiguousarray(Wih[d].T).astype(bf16)       # [in, 4H]
        whhT = np.ascontiguousarray(Whh[d].T).astype(bf16)       # [H, 4H]
        bias = np.ascontiguousarray(b[d].reshape(MT, 128).T).astype(np.float32)
        out.append((wihT, whhT, bias))
    return out


def _launch(nc, in_maps, trace, label):
    from concourse.bass_utils import run_bass_kernel_spmd

    if trace:
        import tempfile
        tmpdir = tempfile.mkdtemp(prefix=f"lstm_trace_{label}_")
        res = run_bass_kernel_spmd(
            nc, in_maps, core_ids=list(range(N_CORES)), trace=True, tmpdir=tmpdir
        )
        last_exec_ns.append(res.exec_time_ns)
        last_trace_dirs.append(tmpdir)
    else:
        res = run_bass_kernel_spmd(nc, in_maps, core_ids=list(range(N_CORES)))
    return res.results


def kernel(**inputs):
    import ml_dtypes

    bf16 = ml_dtypes.bfloat16
    trace = os.environ.get("BASS_LSTM_TRACE", "0") == "1"
    last_exec_ns.clear()
    last_trace_dirs.clear()

    F = np.asarray(inputs["F"]).astype(np.int64)          # [S, M]
    F_lens = np.asarray(inputs["F_lens"]).astype(np.int64)  # [M]
    emb = np.asarray(inputs["emb"], dtype=np.float32)     # [V, E]
    w0 = _prep_weights(np.asarray(inputs["Wih0"], np.float32),
                       np.asarray(inputs["Whh0"], np.float32),
                       np.asarray(inputs["b0"], np.float32), bf16)
    w1 = _prep_weights(np.asarray(inputs["Wih1"], np.float32),
                       np.asarray(inputs["Whh1"], np.float32),
                       np.asarray(inputs["b1"], np.float32), bf16)

    # wm row: -FORCE on i gate rows, +FORCE on f gate rows (gate dim major 1024)
    wm_row = np.zeros((1, 1024), np.float32)
    wm_row[0, :512] = -FORCE
    wm_row[0, 512:] = FORCE
    wm_row = wm_row.astype(bf16)

    nc = _get_program()

    # per-core setup: core 2i fwd / 2i+1 bwd on batch slice i.
    # Scan-order global position: fwd u = s, bwd u = S-1-s; masked iff u >= len.
    valid = np.arange(S)[:, None] < F_lens[None, :]       # [S, M] fwd sense
    x_full = emb[F]                                        # [S, M, E] f32
    x_full = x_full * valid[:, :, None]                    # zero masked positions

    core_meta = []
    in_maps0 = []
    for c in range(N_CORES):
        pair, d = c // 2, c % 2
        sl = slice(pair * B, (pair + 1) * B)
        xs = x_full[:, sl, :]                              # [S, B, E]
        vs = valid[:, sl]                                  # [S, B]
        if d == 1:
            xs = xs[::-1]
            vs = vs[::-1]
        mask_flag = (~vs).astype(np.float32)               # 1.0 where frozen
        wihT, whhT, bias_t = w0[d]
        xtc = np.ascontiguousarray(xs.reshape(S * B, E).T).astype(bf16)
        in_maps0.append({
            "xt": xtc,
            "wih": wihT,
            "whh": whhT,
            "bias": bias_t,
            "wm": wm_row,
            "mk": np.ascontiguousarray(mask_flag.reshape(1, S * B)).astype(bf16),
        })
        core_meta.append((pair, d, sl, mask_flag))

    res0 = _launch(nc, in_maps0, trace, "L0")
    # hout [S, HT, 128, B] -> h [S, B, 512]
    h0 = [r["hout"].transpose(0, 3, 1, 2).reshape(S, B, H) for r in res0]

    in_maps1 = []
    for c in range(N_CORES):
        pair, d, sl, mask_flag = core_meta[c]
        hf = h0[2 * pair]          # fwd scan order == global time
        hb = h0[2 * pair + 1]      # bwd scan order (global u = S-1-s)
        if d == 0:
            x1 = np.concatenate([hf, hb[::-1]], axis=2)    # [S, B, 1024]
        else:
            x1 = np.concatenate([hf[::-1], hb], axis=2)
        wihT, whhT, bias_t = w1[d]
        xtc = np.ascontiguousarray(x1.reshape(S * B, 2 * H).T).astype(bf16)
        in_maps1.append({
            "xt": xtc,
            "wih": wihT,
            "whh": whhT,
            "bias": bias_t,
            "wm": wm_row,
            "mk": np.ascontiguousarray(mask_flag.reshape(1, S * B)).astype(bf16),
        })

    res1 = _launch(nc, in_maps1, trace, "L1")
    h1 = [r["hout"].transpose(0, 3, 1, 2).reshape(S, B, H) for r in res1]

    keep = (valid & (F != 0)).astype(np.float32)           # [S, M]
    out = np.zeros((S, M, 2 * H), np.float32)
    for pair in range(4):
        sl = slice(pair * B, (pair + 1) * B)
        k = keep[:, sl][:, :, None]
        out[:, sl, :H] = h1[2 * pair] * k
        out[:, sl, H:] = h1[2 * pair + 1][::-1] * k
    return out


# revision 8
# speedup vs baseline: 1.0349x; 1.0349x over previous
"""Bidirectional 2-layer LSTM encoder (nn_Encoder) on 8 Trainium2 NeuronCores.

Strategy
--------
- 4 batch slices x 2 directions = 8 cores. Core 2i runs the forward
  direction for batch slice i, core 2i+1 the backward direction.
- The backward direction over a ragged batch is mathematically equivalent to
  a forward scan over the globally time-reversed sequence with the validity
  mask applied (state frozen while masked), so one direction-agnostic device
  program serves all cores; the host prepares per-core scan-ordered inputs.
- State freezing is done arithmetically: a K=1 matmul adds -25/+25 times the
  per-token mask flag to the i/f gate pre-activations (sigmoid saturates to
  0/1 to ~1e-9), so the scan itself has no mask/select ops.
- Per layer the device runs a 128-step recurrent scan in a transposed layout
  (gates/state as [128, batch] tiles, PE weight-load bound), with the batched
  input matmul (Gin) chunk-interleaved into the scan so Gin matmuls fill the
  PE gaps left by each step's elementwise dependency chain. Gin chunks live
  in SBUF (no DRAM round-trip).
- Layers are two launches of the same compiled program; the host reshuffles
  h0 (direction un-reversal + concat) between launches and assembles/masks
  the final output.
"""

import os
import numpy as np

S, M, V, E, H = 128, 256, 32000, 1024, 512
PAD_ID = V - 1
N_CORES = 8
B = 64          # batch columns per core (M / 4 pairs)
KT = 8          # k-tiles of the 1024-dim input
HT = 4          # k-tiles of the 512-dim hidden
MT = 16         # m-tiles of the 4H=2048 gate dim
SPC = 4         # scan steps covered per Gin chunk
FD = SPC * B    # Gin matmul free-dim chunk (256)
NCH = S // SPC  # 32 chunks
FORCE = 25.0    # gate-forcing magnitude for masked steps

_cache = {}

# Populated with [exec_time_ns_layer0, exec_time_ns_layer1] when tracing.
last_exec_ns = []
last_trace_dirs = []


def _build_program():
    import concourse.bacc as bacc
    import concourse.tile as tile
    from concourse import mybir

    f32 = mybir.dt.float32
    bf16 = mybir.dt.bfloat16
    Sig = mybir.ActivationFunctionType.Sigmoid
    Tanh = mybir.ActivationFunctionType.Tanh
    add = mybir.AluOpType.add
    mult = mybir.AluOpType.mult

    nc = bacc.Bacc("TRN2", target_bir_lowering=False, debug=False, num_devices=1)

    xt = nc.dram_tensor("xt", [E, S * B], bf16, kind="ExternalInput").ap()
    wih = nc.dram_tensor("wih", [E, 4 * H], bf16, kind="ExternalInput").ap()
    whh = nc.dram_tensor("whh", [H, 4 * H], bf16, kind="ExternalInput").ap()
    bias = nc.dram_tensor("bias", [128, MT], f32, kind="ExternalInput").ap()
    wm = nc.dram_tensor("wm", [1, 1024], bf16, kind="ExternalInput").ap()
    mk = nc.dram_tensor("mk", [1, S * B], bf16, kind="ExternalInput").ap()
    hout = nc.dram_tensor("hout", [S, HT, 128, B], f32, kind="ExternalOutput").ap()

    with tile.TileContext(nc) as tc:
        with (
            tc.tile_pool(name="wp", bufs=1) as wp,
            tc.tile_pool(name="xp", bufs=24) as xp,
            tc.tile_pool(name="gps", bufs=4, space="PSUM") as gps,
            tc.tile_pool(name="gch", bufs=3) as gchp,
            tc.tile_pool(name="sps", bufs=4, space="PSUM") as sps,
            tc.tile_pool(name="gsum", bufs=4) as gsump,
            tc.tile_pool(name="act", bufs=2) as actp,
            tc.tile_pool(name="st", bufs=2) as stp,
            tc.tile_pool(name="tmp", bufs=2) as tmpp,
        ):
            # ---- resident weights ----
            wih_sb = wp.tile([128, KT * 4 * H], bf16, tag="wih")
            for k in range(KT):
                nc.sync.dma_start(
                    wih_sb[:, k * 4 * H : (k + 1) * 4 * H],
                    wih[k * 128 : (k + 1) * 128, :],
                )
            whh_sb = wp.tile([128, HT * 4 * H], bf16, tag="whh")
            for k in range(HT):
                nc.sync.dma_start(
                    whh_sb[:, k * 4 * H : (k + 1) * 4 * H],
                    whh[k * 128 : (k + 1) * 128, :],
                )
            bias_sb = wp.tile([128, MT], f32, tag="bias")
            nc.sync.dma_start(bias_sb[:], bias[:])
            wm_sb = wp.tile([1, 1024], bf16, tag="wm")
            nc.sync.dma_start(wm_sb[:], wm[:])
            mk_sb = wp.tile([1, S * B], bf16, tag="mk")
            nc.sync.dma_start(mk_sb[:], mk[:])

            xtiles = {}   # chunk -> list of 8 sbuf tiles
            gchunks = {}  # chunk -> sbuf tile [128, MT*FD]

            def emit_xdma(c):
                ts = []
                for k in range(KT):
                    x_t = xp.tile([128, FD], bf16, tag="x")
                    nc.sync.dma_start(
                        x_t[:], xt[k * 128 : (k + 1) * 128, c * FD : (c + 1) * FD]
                    )
                    ts.append(x_t)
                xtiles[c] = ts

            def emit_gin_quarter(c, q):
                # compute Gin for chunk c, m-tiles 4q..4q+3
                if q == 0:
                    gchunks[c] = gchp.tile(
                        [128, MT * FD], f32, tag="gch", name=f"gch{c}"
                    )
                gt = gchunks[c]
                xts = xtiles[c]
                for m in range(4 * q, 4 * q + 4):
                    ps = gps.tile([128, FD], f32, tag="gps")
                    for k in range(KT):
                        nc.tensor.matmul(
                            out=ps[:],
                            lhsT=wih_sb[:, k * 4 * H + m * 128 : k * 4 * H + (m + 1) * 128],
                            rhs=xts[k][:],
                            start=(k == 0),
                            stop=(k == KT - 1 and m >= 8),
                        )
                    if m < 8:
                        # mask forcing on i/f gates
                        nc.tensor.matmul(
                            out=ps[:],
                            lhsT=wm_sb[:, m * 128 : (m + 1) * 128],
                            rhs=mk_sb[:, c * FD : (c + 1) * FD],
                            start=False,
                            stop=True,
                        )
                    nc.vector.tensor_scalar(
                        out=gt[:, m * FD : (m + 1) * FD], in0=ps[:],
                        scalar1=bias_sb[:, m : m + 1], scalar2=None, op0=add,
                    )

            # ---- scan state ----
            c_prev = stp.tile([128, 4 * B], f32, tag="c")
            nc.vector.memset(c_prev[:], 0.0)
            hbf_prev = stp.tile([128, 4 * B], bf16, tag="hbf")
            nc.vector.memset(hbf_prev[:], 0.0)

            def emit_scan_step(s):
                nonlocal c_prev, hbf_prev
                blk, so = s // SPC, s % SPC
                gt = gchunks[blk]
                # gin view: [128, m, so, j] -> per-step slice [128, m-range, j]
                gv = gt[:].rearrange("p (m s j) -> p m s j", m=MT, s=SPC)
                psA = sps.tile([128, 8 * B], f32, tag="ps")
                psB = sps.tile([128, 8 * B], f32, tag="ps")
                for m in range(MT):
                    ps = psA if m < 8 else psB
                    col = (m % 8) * B
                    for k in range(HT):
                        nc.tensor.matmul(
                            out=ps[:, col : col + B],
                            lhsT=whh_sb[:, k * 4 * H + m * 128 : k * 4 * H + (m + 1) * 128],
                            rhs=hbf_prev[:, k * B : (k + 1) * B],
                            start=(k == 0),
                            stop=(k == HT - 1),
                        )
                gsA = gsump.tile([128, 8 * B], f32, tag="gs")
                gsB = gsump.tile([128, 8 * B], f32, tag="gs")
                nc.vector.tensor_tensor(
                    out=gsA[:].rearrange("p (m j) -> p m j", m=8),
                    in0=psA[:].rearrange("p (m j) -> p m j", m=8),
                    in1=gv[:, 0:8, so, :], op=add,
                )
                nc.vector.tensor_tensor(
                    out=gsB[:].rearrange("p (m j) -> p m j", m=8),
                    in0=psB[:].rearrange("p (m j) -> p m j", m=8),
                    in1=gv[:, 8:16, so, :], op=add,
                )

                si = actp.tile([128, 4 * B], f32, tag="si")
                nc.scalar.activation(si[:], gsA[:, : 4 * B], Sig)
                sf = actp.tile([128, 4 * B], f32, tag="sf")
                nc.scalar.activation(sf[:], gsA[:, 4 * B :], Sig)
                tg = actp.tile([128, 4 * B], f32, tag="tg")
                nc.scalar.activation(tg[:], gsB[:, : 4 * B], Tanh)
                so_ = actp.tile([128, 4 * B], f32, tag="so")
                nc.scalar.activation(so_[:], gsB[:, 4 * B :], Sig)

                u = tmpp.tile([128, 4 * B], f32, tag="u")
                nc.vector.tensor_tensor(out=u[:], in0=si[:], in1=tg[:], op=mult)
                cf = tmpp.tile([128, 4 * B], f32, tag="cf")
                nc.vector.tensor_tensor(out=cf[:], in0=sf[:], in1=c_prev[:], op=mult)
                c_new = stp.tile([128, 4 * B], f32, tag="c")
                nc.vector.tensor_tensor(out=c_new[:], in0=cf[:], in1=u[:], op=add)
                tcn = tmpp.tile([128, 4 * B], f32, tag="tcn")
                nc.scalar.activation(tcn[:], c_new[:], Tanh)
                hbf_new = stp.tile([128, 4 * B], bf16, tag="hbf")
                nc.vector.tensor_tensor(out=hbf_new[:], in0=so_[:], in1=tcn[:], op=mult)
                h_new = tmpp.tile([128, 4 * B], f32, tag="h")
                nc.gpsimd.tensor_tensor(out=h_new[:], in0=so_[:], in1=tcn[:], op=mult)

                dst = hout[s, :, :, :].rearrange("k p j -> p k j")
                nc.sync.dma_start(dst, h_new[:].rearrange("p (k j) -> p k j", k=HT))

                c_prev = c_new
                hbf_prev = hbf_new

            # ---- emission: prologue then interleaved steady state ----
            emit_xdma(0)
            emit_xdma(1)
            emit_xdma(2)
            for c in (0, 1):
                for q in range(4):
                    emit_gin_quarter(c, q)
            for s in range(S):
                emit_scan_step(s)
                blk, q = s // SPC, s % SPC
                cc = blk + 2
                if cc < NCH:
                    if q == 0 and cc + 1 < NCH:
                        emit_xdma(cc + 1)
                    emit_gin_quarter(cc, q)

    nc.compile()
    return nc


def _get_program():
    if "nc" not in _cache:
        _cache["nc"] = _build_program()
    return _cache["nc"]


def _prep_weights(Wih, Whh, b, bf16):
    """Per-direction device weight tensors."""
    out = []
    for d in range(2):
        wihT = np.ascontiguousarray(Wih[d].T).astype(bf16)       # [in, 4H]
        whhT = np.ascontiguousarray(Whh[d].T).astype(bf16)       # [H, 4H]
        bias = np.ascontiguousarray(b[d].reshape(MT, 128).T).astype(np.float32)
        out.append((wihT, whhT, bias))
    return out


def _launch(nc, in_maps, trace, label):
    from concourse.bass_utils import run_bass_kernel_spmd

    if trace:
        import tempfile
        tmpdir = tempfile.mkdtemp(prefix=f"lstm_trace_{label}_")
        res = run_bass_kernel_spmd(
            nc, in_maps, core_ids=list(range(N_CORES)), trace=True, tmpdir=tmpdir
        )
        last_exec_ns.append(res.exec_time_ns)
        last_trace_dirs.append(tmpdir)
    else:
        res = run_bass_kernel_spmd(nc, in_maps, core_ids=list(range(N_CORES)))
    return res.results


def kernel(**inputs):
    import ml_dtypes

    bf16 = ml_dtypes.bfloat16
    trace = os.environ.get("BASS_LSTM_TRACE", "0") == "1"
    last_exec_ns.clear()
    last_trace_dirs.clear()

    F = np.asarray(inputs["F"]).astype(np.int64)          # [S, M]
    F_lens = np.asarray(inputs["F_lens"]).astype(np.int64)  # [M]
    emb = np.asarray(inputs["emb"], dtype=np.float32)     # [V, E]
    w0 = _prep_weights(np.asarray(inputs["Wih0"], np.float32),
                       np.asarray(inputs["Whh0"], np.float32),
                       np.asarray(inputs["b0"], np.float32), bf16)
    w1 = _prep_weights(np.asarray(inputs["Wih1"], np.float32),
                       np.asarray(inputs["Whh1"], np.float32),
                       np.asarray(inputs["b1"], np.float32), bf16)

    # wm row: -FORCE on i gate rows, +FORCE on f gate rows (gate dim major 1024)
    wm_row = np.zeros((1, 1024), np.float32)
    wm_row[0, :512] = -FORCE
    wm_row[0, 512:] = FORCE
    wm_row = wm_row.astype(bf16)

    nc = _get_program()

    # per-core setup: core 2i fwd / 2i+1 bwd on batch slice i.
    # Scan-order global position: fwd u = s, bwd u = S-1-s; masked iff u >= len.
    valid = np.arange(S)[:, None] < F_lens[None, :]       # [S, M] fwd sense
    x_full = emb[F]                                        # [S, M, E] f32
    x_full = x_full * valid[:, :, None]                    # zero masked positions

    core_meta = []
    in_maps0 = []
    for c in range(N_CORES):
        pair, d = c // 2, c % 2
        sl = slice(pair * B, (pair + 1) * B)
        xs = x_full[:, sl, :]                              # [S, B, E]
        vs = valid[:, sl]                                  # [S, B]
        if d == 1:
            xs = xs[::-1]
            vs = vs[::-1]
        mask_flag = (~vs).astype(np.float32)               # 1.0 where frozen
        wihT, whhT, bias_t = w0[d]
        xtc = np.ascontiguousarray(xs.reshape(S * B, E).T).astype(bf16)
        in_maps0.append({
            "xt": xtc,
            "wih": wihT,
            "whh": whhT,
            "bias": bias_t,
            "wm": wm_row,
            "mk": np.ascontiguousarray(mask_flag.reshape(1, S * B)).astype(bf16),
        })
        core_meta.append((pair, d, sl, mask_flag))

    res0 = _launch(nc, in_maps0, trace, "L0")
    # hout [S, HT, 128, B] -> h [S, B, 512]
    h0 = [r["hout"].transpose(0, 3, 1, 2).reshape(S, B, H) for r in res0]

    in_maps1 = []
    for c in range(N_CORES):
        pair, d, sl, mask_flag = core_meta[c]
        hf = h0[2 * pair]          # fwd scan order == global time
        hb = h0[2 * pair + 1]      # bwd scan order (global u = S-1-s)
        if d == 0:
            x1 = np.concatenate([hf, hb[::-1]], axis=2)    # [S, B, 1024]
        else:
            x1 = np.concatenate([hf[::-1], hb], axis=2)
        wihT, whhT, bias_t = w1[d]
        xtc = np.ascontiguousarray(x1.reshape(S * B, 2 * H).T).astype(bf16)
        in_maps1.append({
            "xt": xtc,
            "wih": wihT,
            "whh": whhT,
            "bias": bias_t,
            "wm": wm_row,
            "mk": np.ascontiguousarray(mask_flag.reshape(1, S * B)).astype(bf16),
        })

    res1 = _launch(nc, in_maps1, trace, "L1")
    h1 = [r["hout"].transpose(0, 3, 1, 2).reshape(S, B, H) for r in res1]

    keep = (valid & (F != 0)).astype(np.float32)           # [S, M]
    out = np.zeros((S, M, 2 * H), np.float32)
    for pair in range(4):
        sl = slice(pair * B, (pair + 1) * B)
        k = keep[:, sl][:, :, None]
        out[:, sl, :H] = h1[2 * pair] * k
        out[:, sl, H:] = h1[2 * pair + 1][::-1] * k
    return out


# revision 12
# speedup vs baseline: 1.0414x; 1.0063x over previous
"""Bidirectional 2-layer LSTM encoder (nn_Encoder) on 8 Trainium2 NeuronCores.

Strategy
--------
- 4 batch slices x 2 directions = 8 cores. Core 2i runs the forward
  direction for batch slice i, core 2i+1 the backward direction.
- The backward direction over a ragged batch is mathematically equivalent to
  a forward scan over the globally time-reversed sequence with the validity
  mask applied (state frozen while masked), so one direction-agnostic device
  program serves all cores; the host prepares per-core scan-ordered inputs.
- State freezing is done arithmetically: a K=1 matmul adds -25/+25 times the
  per-token mask flag to the i/f gate pre-activations (sigmoid saturates to
  0/1 to ~1e-9), so the scan itself has no mask/select ops.
- Per layer the device runs a 128-step recurrent scan in a transposed layout
  (gates/state as [128, batch] tiles, PE weight-load bound), with the batched
  input matmul (Gin) chunk-interleaved into the scan so Gin matmuls fill the
  PE gaps left by each step's elementwise dependency chain. Gin chunks live
  in SBUF (no DRAM round-trip).
- Layers are two launches of the same compiled program; the host reshuffles
  h0 (direction un-reversal + concat) between launches and assembles/masks
  the final output.
"""

import os
import numpy as np

S, M, V, E, H = 128, 256, 32000, 1024, 512
PAD_ID = V - 1
N_CORES = 8
B = 64          # batch columns per core (M / 4 pairs)
KT = 8          # k-tiles of the 1024-dim input
HT = 4          # k-tiles of the 512-dim hidden
MT = 16         # m-tiles of the 4H=2048 gate dim
SPC = 4         # scan steps covered per Gin chunk
FD = SPC * B    # Gin matmul free-dim chunk (256)
NCH = S // SPC  # 32 chunks
FORCE = 25.0    # gate-forcing magnitude for masked steps

_cache = {}

# Populated with [exec_time_ns_layer0, exec_time_ns_layer1] when tracing.
last_exec_ns = []
last_trace_dirs = []


def _build_program():
    import concourse.bacc as bacc
    import concourse.tile as tile
    from concourse import mybir

    f32 = mybir.dt.float32
    bf16 = mybir.dt.bfloat16
    Sig = mybir.ActivationFunctionType.Sigmoid
    Tanh = mybir.ActivationFunctionType.Tanh
    add = mybir.AluOpType.add
    mult = mybir.AluOpType.mult

    nc = bacc.Bacc("TRN2", target_bir_lowering=False, debug=False, num_devices=1)

    xt = nc.dram_tensor("xt", [E, S * B], bf16, kind="ExternalInput").ap()
    wih = nc.dram_tensor("wih", [E, 4 * H], bf16, kind="ExternalInput").ap()
    whh = nc.dram_tensor("whh", [H, 4 * H], bf16, kind="ExternalInput").ap()
    bias = nc.dram_tensor("bias", [128, MT], f32, kind="ExternalInput").ap()
    wm = nc.dram_tensor("wm", [1, 1024], bf16, kind="ExternalInput").ap()
    mk = nc.dram_tensor("mk", [1, S * B], bf16, kind="ExternalInput").ap()
    hout = nc.dram_tensor("hout", [S, HT, 128, B], f32, kind="ExternalOutput").ap()

    with tile.TileContext(nc) as tc:
        with (
            tc.tile_pool(name="wp", bufs=1) as wp,
            tc.tile_pool(name="xp", bufs=24) as xp,
            tc.tile_pool(name="gps", bufs=6, space="PSUM") as gps,
            tc.tile_pool(name="gch", bufs=3) as gchp,
            tc.tile_pool(name="sps", bufs=2, space="PSUM") as sps,
            tc.tile_pool(name="gsum", bufs=4) as gsump,
            tc.tile_pool(name="act", bufs=2) as actp,
            tc.tile_pool(name="st", bufs=2) as stp,
            tc.tile_pool(name="tmp", bufs=2) as tmpp,
        ):
            # ---- resident weights ----
            wih_sb = wp.tile([128, KT * 4 * H], bf16, tag="wih")
            for k in range(KT):
                nc.sync.dma_start(
                    wih_sb[:, k * 4 * H : (k + 1) * 4 * H],
                    wih[k * 128 : (k + 1) * 128, :],
                )
            bias_sb = wp.tile([128, MT], f32, tag="bias")
            nc.sync.dma_start(bias_sb[:], bias[:])
            wm_sb = wp.tile([1, 1024], bf16, tag="wm")
            nc.sync.dma_start(wm_sb[:], wm[:])
            mk_sb = wp.tile([1, S * B], bf16, tag="mk")
            nc.sync.dma_start(mk_sb[:], mk[:])
            whh_sb = wp.tile([128, HT * 4 * H], bf16, tag="whh")

            xtiles = {}   # chunk -> list of 8 sbuf tiles
            gchunks = {}  # chunk -> sbuf tile [128, MT*FD]

            def emit_xdma(c):
                ts = []
                for k in range(KT):
                    x_t = xp.tile([128, FD], bf16, tag="x")
                    nc.sync.dma_start(
                        x_t[:], xt[k * 128 : (k + 1) * 128, c * FD : (c + 1) * FD]
                    )
                    ts.append(x_t)
                xtiles[c] = ts

            def emit_gin_quarter(c, q):
                # compute Gin for chunk c, m-tiles 4q..4q+3
                if q == 0:
                    gchunks[c] = gchp.tile(
                        [128, MT * FD], f32, tag="gch", name=f"gch{c}"
                    )
                gt = gchunks[c]
                xts = xtiles[c]
                for m in range(4 * q, 4 * q + 4):
                    ps = gps.tile([128, FD], f32, tag="gps")
                    for k in range(KT):
                        nc.tensor.matmul(
                            out=ps[:],
                            lhsT=wih_sb[:, k * 4 * H + m * 128 : k * 4 * H + (m + 1) * 128],
                            rhs=xts[k][:],
                            start=(k == 0),
                            stop=(k == KT - 1 and m >= 8),
                        )
                    if m < 8:
                        # mask forcing on i/f gates
                        nc.tensor.matmul(
                            out=ps[:],
                            lhsT=wm_sb[:, m * 128 : (m + 1) * 128],
                            rhs=mk_sb[:, c * FD : (c + 1) * FD],
                            start=False,
                            stop=True,
                        )
                    nc.vector.tensor_scalar(
                        out=gt[:, m * FD : (m + 1) * FD], in0=ps[:],
                        scalar1=bias_sb[:, m : m + 1], scalar2=None, op0=add,
                    )

            # ---- scan state ----
            c_prev = stp.tile([128, 4 * B], f32, tag="c")
            nc.vector.memset(c_prev[:], 0.0)
            hbf_prev = stp.tile([128, 4 * B], bf16, tag="hbf")
            nc.vector.memset(hbf_prev[:], 0.0)

            def emit_scan_step(s):
                nonlocal c_prev, hbf_prev
                blk, so = s // SPC, s % SPC
                gt = gchunks[blk]
                # gin view: [128, m, so, j] -> per-step slice [128, m-range, j]
                gv = gt[:].rearrange("p (m s j) -> p m s j", m=MT, s=SPC)
                psA = sps.tile([128, 8 * B], f32, tag="ps")
                psB = sps.tile([128, 8 * B], f32, tag="ps")
                for m in range(MT):
                    ps = psA if m < 8 else psB
                    col = (m % 8) * B
                    for k in range(HT):
                        nc.tensor.matmul(
                            out=ps[:, col : col + B],
                            lhsT=whh_sb[:, k * 4 * H + m * 128 : k * 4 * H + (m + 1) * 128],
                            rhs=hbf_prev[:, k * B : (k + 1) * B],
                            start=(k == 0),
                            stop=(k == HT - 1),
                        )
                gsA = gsump.tile([128, 8 * B], f32, tag="gs")
                gsB = gsump.tile([128, 8 * B], f32, tag="gs")
                nc.vector.tensor_tensor(
                    out=gsA[:].rearrange("p (m j) -> p m j", m=8),
                    in0=psA[:].rearrange("p (m j) -> p m j", m=8),
                    in1=gv[:, 0:8, so, :], op=add,
                )
                nc.vector.tensor_tensor(
                    out=gsB[:].rearrange("p (m j) -> p m j", m=8),
                    in0=psB[:].rearrange("p (m j) -> p m j", m=8),
                    in1=gv[:, 8:16, so, :], op=add,
                )

                sif = actp.tile([128, 8 * B], f32, tag="sif")
                nc.scalar.activation(sif[:], gsA[:], Sig)
                tg = actp.tile([128, 4 * B], f32, tag="tg")
                nc.scalar.activation(tg[:], gsB[:, : 4 * B], Tanh)
                so_ = actp.tile([128, 4 * B], f32, tag="so")
                nc.scalar.activation(so_[:], gsB[:, 4 * B :], Sig)

                u = tmpp.tile([128, 4 * B], f32, tag="u")
                nc.vector.tensor_tensor(out=u[:], in0=sif[:, : 4 * B], in1=tg[:], op=mult)
                cf = tmpp.tile([128, 4 * B], f32, tag="cf")
                nc.vector.tensor_tensor(out=cf[:], in0=sif[:, 4 * B :], in1=c_prev[:], op=mult)
                c_new = stp.tile([128, 4 * B], f32, tag="c")
                nc.vector.tensor_tensor(out=c_new[:], in0=cf[:], in1=u[:], op=add)
                tcn = tmpp.tile([128, 4 * B], f32, tag="tcn")
                nc.scalar.activation(tcn[:], c_new[:], Tanh)
                hbf_new = stp.tile([128, 4 * B], bf16, tag="hbf")
                nc.vector.tensor_tensor(out=hbf_new[:], in0=so_[:], in1=tcn[:], op=mult)
                h_new = tmpp.tile([128, 4 * B], f32, tag="h")
                nc.gpsimd.tensor_tensor(out=h_new[:], in0=so_[:], in1=tcn[:], op=mult)

                dst = hout[s, :, :, :].rearrange("k p j -> p k j")
                nc.sync.dma_start(dst, h_new[:].rearrange("p (k j) -> p k j", k=HT))

                c_prev = c_new
                hbf_prev = hbf_new

            # ---- emission: prologue then interleaved steady state ----
            emit_xdma(0)
            emit_xdma(1)
            emit_xdma(2)
            for k in range(HT):
                nc.sync.dma_start(
                    whh_sb[:, k * 4 * H : (k + 1) * 4 * H],
                    whh[k * 128 : (k + 1) * 128, :],
                )
            for c in (0, 1):
                for q in range(4):
                    emit_gin_quarter(c, q)
            for s in range(S):
                emit_scan_step(s)
                blk, q = s // SPC, s % SPC
                cc = blk + 2
                if cc < NCH:
                    if q == 0 and cc + 1 < NCH:
                        emit_xdma(cc + 1)
                    emit_gin_quarter(cc, q)

    nc.compile()
    return nc


def _get_program():
    if "nc" not in _cache:
        _cache["nc"] = _build_program()
    return _cache["nc"]


def _prep_weights(Wih, Whh, b, bf16):
    """Per-direction device weight tensors."""
    out = []
    for d in range(2):
        wihT = np.ascontiguousarray(Wih[d].T).astype(bf16)       # [in, 4H]
        whhT = np.ascontiguousarray(Whh[d].T).astype(bf16)       # [H, 4H]
        bias = np.ascontiguousarray(b[d].reshape(MT, 128).T).astype(np.float32)
        out.append((wihT, whhT, bias))
    return out


def _launch(nc, in_maps, trace, label):
    from concourse.bass_utils import run_bass_kernel_spmd

    if trace:
        import tempfile
        tmpdir = tempfile.mkdtemp(prefix=f"lstm_trace_{label}_")
        res = run_bass_kernel_spmd(
            nc, in_maps, core_ids=list(range(N_CORES)), trace=True, tmpdir=tmpdir
        )
        last_exec_ns.append(res.exec_time_ns)
        last_trace_dirs.append(tmpdir)
    else:
        res = run_bass_kernel_spmd(nc, in_maps, core_ids=list(range(N_CORES)))
    return res.results


def kernel(**inputs):
    import ml_dtypes

    bf16 = ml_dtypes.bfloat16
    trace = os.environ.get("BASS_LSTM_TRACE", "0") == "1"
    last_exec_ns.clear()
    last_trace_dirs.clear()

    F = np.asarray(inputs["F"]).astype(np.int64)          # [S, M]
    F_lens = np.asarray(inputs["F_lens"]).astype(np.int64)  # [M]
    emb = np.asarray(inputs["emb"], dtype=np.float32)     # [V, E]
    w0 = _prep_weights(np.asarray(inputs["Wih0"], np.float32),
                       np.asarray(inputs["Whh0"], np.float32),
                       np.asarray(inputs["b0"], np.float32), bf16)
    w1 = _prep_weights(np.asarray(inputs["Wih1"], np.float32),
                       np.asarray(inputs["Whh1"], np.float32),
                       np.asarray(inputs["b1"], np.float32), bf16)

    # wm row: -FORCE on i gate rows, +FORCE on f gate rows (gate dim major 1024)
    wm_row = np.zeros((1, 1024), np.float32)
    wm_row[0, :512] = -FORCE
    wm_row[0, 512:] = FORCE
    wm_row = wm_row.astype(bf16)

    nc = _get_program()

    # per-core setup: core 2i fwd / 2i+1 bwd on batch slice i.
    # Scan-order global position: fwd u = s, bwd u = S-1-s; masked iff u >= len.
    valid = np.arange(S)[:, None] < F_lens[None, :]       # [S, M] fwd sense
    x_full = emb[F]                                        # [S, M, E] f32
    x_full = x_full * valid[:, :, None]                    # zero masked positions

    core_meta = []
    in_maps0 = []
    for c in range(N_CORES):
        pair, d = c // 2, c % 2
        sl = slice(pair * B, (pair + 1) * B)
        xs = x_full[:, sl, :]                              # [S, B, E]
        vs = valid[:, sl]                                  # [S, B]
        if d == 1:
            xs = xs[::-1]
            vs = vs[::-1]
        mask_flag = (~vs).astype(np.float32)               # 1.0 where frozen
        wihT, whhT, bias_t = w0[d]
        xtc = np.ascontiguousarray(xs.reshape(S * B, E).T).astype(bf16)
        in_maps0.append({
            "xt": xtc,
            "wih": wihT,
            "whh": whhT,
            "bias": bias_t,
            "wm": wm_row,
            "mk": np.ascontiguousarray(mask_flag.reshape(1, S * B)).astype(bf16),
        })
        core_meta.append((pair, d, sl, mask_flag))

    res0 = _launch(nc, in_maps0, trace, "L0")
    # hout [S, HT, 128, B] -> h [S, B, 512]
    h0 = [r["hout"].transpose(0, 3, 1, 2).reshape(S, B, H) for r in res0]

    in_maps1 = []
    for c in range(N_CORES):
        pair, d, sl, mask_flag = core_meta[c]
        hf = h0[2 * pair]          # fwd scan order == global time
        hb = h0[2 * pair + 1]      # bwd scan order (global u = S-1-s)
        if d == 0:
            x1 = np.concatenate([hf, hb[::-1]], axis=2)    # [S, B, 1024]
        else:
            x1 = np.concatenate([hf[::-1], hb], axis=2)
        wihT, whhT, bias_t = w1[d]
        xtc = np.ascontiguousarray(x1.reshape(S * B, 2 * H).T).astype(bf16)
        in_maps1.append({
            "xt": xtc,
            "wih": wihT,
            "whh": whhT,
            "bias": bias_t,
            "wm": wm_row,
            "mk": np.ascontiguousarray(mask_flag.reshape(1, S * B)).astype(bf16),
        })

    res1 = _launch(nc, in_maps1, trace, "L1")
    h1 = [r["hout"].transpose(0, 3, 1, 2).reshape(S, B, H) for r in res1]

    keep = (valid & (F != 0)).astype(np.float32)           # [S, M]
    out = np.zeros((S, M, 2 * H), np.float32)
    for pair in range(4):
        sl = slice(pair * B, (pair + 1) * B)
        k = keep[:, sl][:, :, None]
        out[:, sl, :H] = h1[2 * pair] * k
        out[:, sl, H:] = h1[2 * pair + 1][::-1] * k
    return out
